# revision 1
# baseline (speedup 1.0000x reference)
"""Trainium2 Bass kernel for nn_BoundaryLoss (8-core SPMD).

Self-contained: builds the Bass module once, shards full inputs across 8
NeuronCores (data-parallel over batch for the mask/gather stage; anchors x
pos x neg pairwise loss sharded by pos-memory columns), runs via
concourse.bass_utils.run_bass_kernel_spmd, and sums the per-core partial
losses on the host.
"""

import json
import sys
import types
import contextlib
import ctypes

import numpy as np

# ---------------------------------------------------------------------------
# Workaround 1: the walrus compiler in this container accepts only ONE sync
# wait per instruction; Tile's scheduler emits several on join points.  Split
# extra waits into standalone wait-only EventSemaphore instructions inserted
# right before the owning instruction (same engine, same block).
# ---------------------------------------------------------------------------


def _split_multiwaits_json(bir_bytes: bytes) -> bytes:
    j = json.loads(bir_bytes)
    ctr = 0
    changed = False
    for f in j.get("functions", []):
        for bb in f.get("blocks", []):
            new_insts = []
            for inst in bb.get("instructions", []):
                si = inst.get("sync_info")
                ow = (si or {}).get("on_wait") or []
                if len(ow) > 1:
                    changed = True
                    for w in ow[:-1]:
                        ctr += 1
                        new_insts.append(
                            {
                                "debug": inst.get("debug", 0),
                                "engine": inst["engine"],
                                "ins": [],
                                "outs": [],
                                "name": f"I-wsplit-{ctr}",
                                "opcode": "EventSemaphore",
                                "sync_info": {"on_update": [], "on_wait": [w]},
                            }
                        )
                    si["on_wait"] = [ow[-1]]
                new_insts.append(inst)
            bb["instructions"] = new_insts
    if not changed:
        return bir_bytes
    return json.dumps(j).encode()


_patched = False


def _install_patches():
    global _patched
    if _patched:
        return
    from concourse import bass as _bass

    _orig = _bass.Bass.to_json_bytes

    def _to_json_bytes(self):
        return _split_multiwaits_json(_orig(self))

    _bass.Bass.to_json_bytes = _to_json_bytes

    # Workaround 3: EVENT_SEMAPHORE_RANGE_CLEAR encodes a variable-length
    # struct this walrus only accepts for small ranges; clear in chunks.
    from concourse.bass import SemaphoreHandle as _SH, compact_to_ranges as _ctr

    def _clear_and_free(self, sems):
        if not sems:
            return
        sem_nums = [s.num if isinstance(s, _SH) else s for s in sems]
        for sem_range in _ctr(sem_nums):
            assert self._state.free_isdisjoint(sem_range)
            lo = sem_range.start
            while lo < sem_range.stop:
                hi = min(lo + 3, sem_range.stop)
                sub = range(lo, hi)
                self.gpsimd.dma_reset(sub)
                self.gpsimd.sem_clear(sub)
                lo = hi
        self._state.prepend_free_semaphores(sem_nums)
        for poison_set in self._tile_sem_poison_stack:
            poison_set.update(sem_nums)

    _bass.Bass.clear_and_free_semaphores = _clear_and_free

    # Workaround 2: the image lacks antenv.axon_hooks, so trace=True (NTFF
    # profiling) silently degrades.  Provide the module and register the
    # ctypes hook from trn_agent_boot if available.
    try:
        import antenv

        if "antenv.axon_hooks" not in sys.modules:
            m = types.ModuleType("antenv.axon_hooks")
            _store = {}
            m.set_axon_ntff_profile_hook = lambda h: _store.__setitem__("h", h)
            m.get_axon_ntff_profile_hook = lambda: _store.get("h")
            sys.modules["antenv.axon_hooks"] = m
            antenv.axon_hooks = m
            try:
                from trn_agent_boot.trn_boot import _ntff_profile_via_ctypes

                m.set_axon_ntff_profile_hook(
                    _ntff_profile_via_ctypes("/opt/axon/libaxon_pjrt.so")
                )
            except Exception:
                pass
    except Exception:
        pass
    _patched = True


# ---------------------------------------------------------------------------
# Problem constants (hardcoded from the spec)
# ---------------------------------------------------------------------------
B, C, H, W = 8, 21, 512, 512
hh = ww = 128
D = 128
M = 1000
KP = M // 3  # 333
KA = M // 10  # 100
MARGIN = 0.2
NPIX = hh * ww  # 16384 per core
NCORES = 8
PCOLS = M // NCORES  # 125 pos-columns per core

# contribution table layout (rows)
ANC0, ANCW = 0, 256
POS0, POSW = 256, 720
NEG0, NEGW = 976, 720
CTOT = 1696

DVE_COLS = 54  # pairwise columns on DVE; rest on ScalarE

TRACE = False
LAST_EXEC_NS = None

_cache = {}


def _build_module():
    from concourse import bass, tile
    import concourse.mybir as mybir

    dt = mybir.dt
    F32 = dt.float32
    F16 = dt.float16
    Alu = mybir.AluOpType
    Act = mybir.ActivationFunctionType

    nc = bass.Bass(
        trn_type="TRN2", target_bir_lowering=False, debug=False, num_devices=NCORES
    )

    # ---- I/O ----
    preds_t = nc.dram_tensor("preds_t", [128, C * 128], F32, kind="ExternalInput").ap()
    gts_t = nc.dram_tensor("gts_t", [128, 128], dt.int32, kind="ExternalInput").ap()
    embp = nc.dram_tensor("embp", [NPIX, D], F32, kind="ExternalInput").ap()
    posmem = nc.dram_tensor("posmem", [M, D], F32, kind="ExternalInput").ap()
    negmem = nc.dram_tensor("negmem", [M, D], F32, kind="ExternalInput").ap()
    trils_in = nc.dram_tensor("trils", [128, 128], F32, kind="ExternalInput").ap()
    ident_in = nc.dram_tensor("ident", [128, 128], F32, kind="ExternalInput").ap()
    rowiota_in = nc.dram_tensor("rowiota", [128, 1], F32, kind="ExternalInput").ap()
    riota1_in = nc.dram_tensor("riota1", [128, 1], F32, kind="ExternalInput").ap()
    siota3_in = nc.dram_tensor("siota3", [128, 3], F32, kind="ExternalInput").ap()
    prefmask_in = nc.dram_tensor("prefmask", [8, 1], F32, kind="ExternalInput").ap()
    kvec_in = nc.dram_tensor("kvec", [1, 4], F32, kind="ExternalInput").ap()
    poff_in = nc.dram_tensor("poff", [1, 1], dt.int32, kind="ExternalInput").ap()
    out_d = nc.dram_tensor("out", [1, 1], F32, kind="ExternalOutput").ap()

    cnt_loc = nc.dram_tensor("cnt_loc", [1, 4], F32).ap()
    cnt_all = nc.dram_tensor("cnt_all", [8, 4], F32, addr_space="Shared").ap()
    contrib = nc.dram_tensor("contrib", [CTOT, D], F32).ap()
    contrib_o = nc.dram_tensor("contrib_o", [CTOT, D], F32, addr_space="Shared").ap()
    possim_d = nc.dram_tensor("possim_d", [KA, M], F32).ap()

    groups = [list(range(NCORES))]

    with tile.TileContext(nc) as tc:
        with tc.tile_pool(name="cst", bufs=1) as cst, \
             tc.tile_pool(name="wk", bufs=2) as wk, \
             tc.tile_pool(name="big", bufs=1) as big, \
             tc.tile_pool(name="ps", bufs=1, space="PSUM") as ps, \
             tc.tile_pool(name="ps2", bufs=1, space="PSUM") as ps2, \
             tc.tile_pool(name="simp", bufs=1, space="PSUM") as simp:

            # ---------- constant / input loads ----------
            P_sb = big.tile([128, C * 128], F32)
            nc.sync.dma_start(P_sb[:], preds_t)
            G = wk.tile([128, 128], dt.int32)
            nc.sync.dma_start(G[:], gts_t)
            trils = cst.tile([128, 128], F32)
            nc.sync.dma_start(trils[:], trils_in)
            ident = cst.tile([128, 128], F32)
            nc.sync.dma_start(ident[:], ident_in)
            rowiota = cst.tile([128, 1], F32)
            nc.sync.dma_start(rowiota[:], rowiota_in)
            riota1 = cst.tile([128, 1], F32)
            nc.sync.dma_start(riota1[:], riota1_in)
            siota3 = cst.tile([128, 3], F32)
            nc.sync.dma_start(siota3[:], siota3_in)
            prefmask = cst.tile([8, 1], F32)
            nc.sync.dma_start(prefmask[:], prefmask_in)
            kvec = cst.tile([1, 4], F32)
            nc.sync.dma_start(kvec[:], kvec_in)
            poff_sb = cst.tile([1, 1], dt.int32)
            nc.sync.dma_start(poff_sb[:], poff_in)

            zeros = cst.tile([128, 128], F32)
            nc.vector.memset(zeros[:], 0.0)
            ones_t = cst.tile([128, 128], F32)
            nc.vector.memset(ones_t[:], 1.0)
            ones_c = cst.tile([128, 1], F32)
            nc.vector.memset(ones_c[:], 1.0)
            ones_r16 = cst.tile([1, 16], F32)
            nc.vector.memset(ones_r16[:], 1.0)
            ones_r128 = cst.tile([1, 128], F32)
            nc.vector.memset(ones_r128[:], 1.0)

            # memory tables (8 chunks of 128 rows; last holds 104)
            pm = []
            nm = []
            for i in range(8):
                r0 = 128 * i
                rn = min(128, M - r0)
                t1 = wk.tile([128, 128], F32, name=f"pm{i}", tag="pmem", bufs=1)
                nc.sync.dma_start(t1[0:rn, :], posmem[r0 : r0 + rn, :])
                pm.append(t1)
                t2 = wk.tile([128, 128], F32, name=f"nm{i}", tag="nmem", bufs=1)
                nc.sync.dma_start(t2[0:rn, :], negmem[r0 : r0 + rn, :])
                nm.append(t2)

            # zero the contribution table
            for i in range(13):
                nc.sync.dma_start(contrib[128 * i : 128 * (i + 1), :], zeros[:])
            nc.sync.dma_start(contrib[1664:1696, :], zeros[0:32, :])

            # ---------- masks (wrap-dense [128,128], f32 0/1) ----------
            mx = wk.tile([128, 128], F32)
            v = P_sb[:, 128 : C * 128].rearrange("p (c f) -> p f c", c=C - 1)
            nc.vector.tensor_reduce(mx[:], v, axis=mybir.AxisListType.X, op=Alu.max)
            predm = wk.tile([128, 128], F32)
            nc.vector.tensor_tensor(out=predm[:], in0=mx[:], in1=P_sb[:, 0:128], op=Alu.is_gt)
            t1m = wk.tile([128, 128], F32)
            nc.vector.tensor_scalar(out=t1m[:], in0=G[:], scalar1=0.0, scalar2=None, op0=Alu.not_equal)
            t2m = wk.tile([128, 128], F32)
            nc.vector.tensor_scalar(out=t2m[:], in0=G[:], scalar1=255.0, scalar2=None, op0=Alu.not_equal)
            e0m = wk.tile([128, 128], F32)
            nc.vector.tensor_scalar(out=e0m[:], in0=G[:], scalar1=0.0, scalar2=None, op0=Alu.is_equal)
            gtm = wk.tile([128, 128], F32)
            nc.vector.tensor_tensor(out=gtm[:], in0=t1m[:], in1=t2m[:], op=Alu.mult)
            npredm = wk.tile([128, 128], F32)
            nc.vector.tensor_scalar(out=npredm[:], in0=predm[:], scalar1=-1.0, scalar2=1.0, op0=Alu.mult, op1=Alu.add)
            anc_m = wk.tile([128, 128], F32, bufs=1)
            nc.vector.tensor_tensor(out=anc_m[:], in0=predm[:], in1=gtm[:], op=Alu.mult)
            pos_m = wk.tile([128, 128], F32, bufs=1)
            nc.vector.tensor_tensor(out=pos_m[:], in0=gtm[:], in1=npredm[:], op=Alu.mult)
            neg_m = wk.tile([128, 128], F32, bufs=1)
            nc.vector.tensor_tensor(out=neg_m[:], in0=predm[:], in1=e0m[:], op=Alu.mult)
            masks = [anc_m, pos_m, neg_m]

            # ---------- local counts -> AllGather ----------
            rs3 = wk.tile([128, 4], F32)
            nc.vector.memset(rs3[:], 0.0)
            for xi, mk in enumerate(masks):
                nc.vector.tensor_reduce(rs3[:, xi : xi + 1], mk[:], axis=mybir.AxisListType.X, op=Alu.add)
            cnt_ps = ps.tile([1, 4], F32, tag="tiny")
            nc.tensor.matmul(cnt_ps[:], ones_c[:], rs3[:], start=True, stop=True)
            cnt_sb = wk.tile([1, 4], F32)
            nc.scalar.copy(cnt_sb[:], cnt_ps[:])
            nc.sync.dma_start(cnt_loc, cnt_sb[:])
            nc.gpsimd.collective_compute(
                "AllGather", Alu.bypass, replica_groups=groups, ins=[cnt_loc], outs=[cnt_all]
            )
            ca = wk.tile([8, 4], F32)
            nc.sync.dma_start(ca[:], cnt_all)

            # ---------- offsets ----------
            g0_ps = ps.tile([1, 4], F32, tag="tiny")
            nc.tensor.matmul(g0_ps[:], prefmask[:], ca[:], start=True, stop=True)
            g0r = wk.tile([1, 4], F32)
            nc.scalar.copy(g0r[:], g0_ps[:])
            tot_ps = ps.tile([1, 4], F32, tag="tiny")
            nc.tensor.matmul(tot_ps[:], ones_c[0:8, :], ca[:], start=True, stop=True)
            totr = wk.tile([1, 4], F32)
            nc.scalar.copy(totr[:], tot_ps[:])
            cntf = wk.tile([1, 4], F32)  # final counts: min(total, k)
            nc.vector.tensor_tensor(out=cntf[:], in0=totr[:], in1=kvec[:], op=Alu.min)
            srow = wk.tile([1, 4], F32)  # S = clamp(k - g0, 0, 384)
            nc.vector.tensor_tensor(out=srow[:], in0=kvec[:], in1=g0r[:], op=Alu.subtract)
            nc.vector.tensor_scalar(out=srow[:], in0=srow[:], scalar1=0.0, scalar2=384.0, op0=Alu.max, op1=Alu.min)
            g0c = wk.tile([1, 4], F32)  # clamped g0
            nc.vector.tensor_tensor(out=g0c[:], in0=g0r[:], in1=kvec[:], op=Alu.min)
            g0c_i = wk.tile([1, 4], dt.int32)
            nc.vector.tensor_copy(g0c_i[:], g0c[:])

            # broadcast counts to [128,1] columns
            cb_ps = ps.tile([128, 4], F32, tag="tiny")
            nc.tensor.matmul(cb_ps[:], ones_r128[:], cntf[:], start=True, stop=True)
            cntb = wk.tile([128, 4], F32)
            nc.scalar.copy(cntb[:], cb_ps[:])
            sb_ps = ps.tile([128, 4], F32, tag="tiny")
            nc.tensor.matmul(sb_ps[:], ones_r128[:], srow[:], start=True, stop=True)
            s128 = wk.tile([128, 4], F32)
            nc.scalar.copy(s128[:], sb_ps[:])

            # ---------- selection per mask (crossing search + indirect gather) ----------
            specs = [
                (anc_m, 1, ANC0, 0),
                (pos_m, 3, POS0, 1),
                (neg_m, 3, NEG0, 2),
            ]
            for mk, ngrp, base, xi in specs:
                scn = wk.tile([128, 128], F32, name=f"scn{xi}", tag="scn")
                nc.vector.tensor_tensor_scan(scn[:], mk[:], zeros[:], 0.0, Alu.add, Alu.add)
                ro_ps = ps2.tile([128, 1], F32, name=f"rops{xi}", tag="pf")
                nc.tensor.matmul(ro_ps[:], trils[:], scn[:, 127:128], start=True, stop=True)
                rowoff = wk.tile([128, 1], F32, name=f"rowoff{xi}", tag="rowoff")
                nc.scalar.copy(rowoff[:], ro_ps[:])
                Pg = wk.tile([128, 128], F32, name=f"Pg{xi}", tag="Pg")
                nc.vector.tensor_scalar(out=Pg[:], in0=scn[:], scalar1=rowoff[:], scalar2=None, op0=Alu.add)
                roT_ps = ps2.tile([128, 128], F32, name=f"roT{xi}", tag="pf")
                nc.tensor.transpose(roT_ps[0:1, :], rowoff[:], ident[:])
                roT = wk.tile([1, 128], F32, name=f"roTs{xi}", tag="roT")
                nc.scalar.copy(roT[:], roT_ps[0:1, :])
                rb_ps = ps2.tile([128, 128], F32, name=f"rb{xi}", tag="pf")
                nc.tensor.matmul(rb_ps[:], ones_r128[:], roT[:], start=True, stop=True)
                RB = wk.tile([128, 128], F32, name=f"RB{xi}", tag="RB")
                nc.scalar.copy(RB[:], rb_ps[:])
                IDXF = wk.tile([128, 4], F32, name=f"IDXF{xi}", tag="IDXF", bufs=1)
                for cch in range(ngrp):
                    sio = siota3[:, cch : cch + 1]
                    cmp1 = wk.tile([128, 128], F32, name=f"cmp1{xi}{cch}", tag="cmp1")
                    nc.vector.tensor_scalar(out=cmp1[:], in0=RB[:], scalar1=sio, scalar2=None, op0=Alu.is_le)
                    rc = wk.tile([128, 1], F32, name=f"rc{xi}{cch}", tag="rc")
                    nc.vector.tensor_reduce(rc[:], cmp1[:], axis=mybir.AxisListType.X, op=Alu.add)
                    rcT_ps = ps2.tile([128, 128], F32, name=f"rcT{xi}{cch}", tag="pf")
                    nc.tensor.transpose(rcT_ps[0:1, :], rc[:], ident[:])
                    rcT = wk.tile([1, 128], F32, name=f"rcTs{xi}{cch}", tag="rcT")
                    nc.scalar.copy(rcT[:], rcT_ps[0:1, :])
                    rcb_ps = ps2.tile([128, 128], F32, name=f"rcb{xi}{cch}", tag="pf")
                    nc.tensor.matmul(rcb_ps[:], ones_r128[:], rcT[:], start=True, stop=True)
                    Omat = wk.tile([128, 128], F32, name=f"O{xi}{cch}", tag="Omat")
                    nc.vector.tensor_scalar(out=Omat[:], in0=rcb_ps[:], scalar1=riota1[:], scalar2=None, op0=Alu.is_equal)
                    prow_ps = ps2.tile([128, 128], F32, name=f"prw{xi}{cch}", tag="pf")
                    nc.tensor.matmul(prow_ps[:], Omat[:], Pg[:], start=True, stop=True)
                    cmp2 = wk.tile([128, 128], F32, name=f"cmp2{xi}{cch}", tag="cmp2")
                    nc.vector.tensor_scalar(out=cmp2[:], in0=prow_ps[:], scalar1=sio, scalar2=None, op0=Alu.is_le)
                    wc = wk.tile([128, 1], F32, name=f"wc{xi}{cch}", tag="wc")
                    nc.vector.tensor_reduce(wc[:], cmp2[:], axis=mybir.AxisListType.X, op=Alu.add)
                    idxc = wk.tile([128, 1], F32, name=f"idxc{xi}{cch}", tag="idxc")
                    nc.vector.tensor_scalar(out=idxc[:], in0=rc[:], scalar1=128.0, scalar2=-128.0, op0=Alu.mult, op1=Alu.add)
                    nc.vector.tensor_tensor(out=idxc[:], in0=idxc[:], in1=wc[:], op=Alu.add)
                    # invalidate slots >= S (oob index -> skipped by bounds check)
                    vkeep = wk.tile([128, 1], F32, name=f"vk{xi}{cch}", tag="vkeep")
                    nc.vector.tensor_scalar(out=vkeep[:], in0=sio, scalar1=s128[:, xi : xi + 1], scalar2=None, op0=Alu.is_lt)
                    nc.vector.tensor_scalar(out=vkeep[:], in0=vkeep[:], scalar1=-50000.0, scalar2=50000.0, op0=Alu.mult, op1=Alu.add)
                    nc.vector.tensor_tensor(out=idxc[:], in0=idxc[:], in1=vkeep[:], op=Alu.add)
                    nc.vector.tensor_copy(IDXF[:, cch : cch + 1], idxc[:])
                ixT_ps = ps2.tile([4, 128], F32, name=f"ixT{xi}", tag="pf")
                nc.tensor.transpose(ixT_ps[0:ngrp, :], IDXF[:, 0:ngrp], ident[:])
                ixT = wk.tile([4, 128], F32, name=f"ixTs{xi}", tag="ixT", bufs=1)
                nc.scalar.copy(ixT[0:ngrp, :], ixT_ps[0:ngrp, :])
                ixTi = wk.tile([4, 128], dt.int32, name=f"ixTi{xi}", tag="ixTi", bufs=1)
                nc.vector.tensor_copy(ixTi[0:ngrp, :], ixT[0:ngrp, :])
                idxrow = wk.tile([1, 384], dt.int32, name=f"idxrow{xi}", tag="idxrow", bufs=1)
                nc.sync.dma_start(idxrow[0:1, 0 : ngrp * 128], ixTi[0:ngrp, :])
                gat = wk.tile([128, ngrp, 128], F32, name=f"gat{xi}", tag="gat", bufs=1)
                nc.vector.memset(gat[:], 0.0)
                nc.gpsimd.indirect_dma_start(
                    out=gat[:],
                    out_offset=None,
                    in_=embp,
                    in_offset=bass.IndirectOffsetOnAxis(ap=idxrow[0:1, 0 : ngrp * 128], axis=0),
                    bounds_check=NPIX - 1,
                    oob_is_err=False,
                )
                # normalize rows (eps 1e-12)
                for g in range(ngrp):
                    gv = gat[:, g, :]
                    ssq = wk.tile([128, 1], F32, name=f"ssq{xi}{g}", tag="ssq")
                    scr0 = wk.tile([128, 128], F32, name=f"scr0{xi}{g}", tag="scr0")
                    nc.vector.scalar_tensor_tensor(out=scr0[:], in0=gv, scalar=1.0, in1=gv, op0=Alu.mult, op1=Alu.mult, accum_out=ssq[:])
                    nc.scalar.sqrt(ssq[:], ssq[:])
                    nc.vector.tensor_scalar(out=ssq[:], in0=ssq[:], scalar1=1e-12, scalar2=None, op0=Alu.max)
                    nc.vector.reciprocal(ssq[:], ssq[:])
                    nc.vector.tensor_scalar(out=gv, in0=gv, scalar1=ssq[:], scalar2=None, op0=Alu.mult)
                g0reg = nc.values_load(g0c_i[0:1, xi : xi + 1].to_broadcast((1, 1)))
                nc.sync.dma_start(contrib[bass.ds(g0reg + base, ngrp * 128), :], gat[:, 0:ngrp, :])

            # ---------- AllReduce contributions ----------
            nc.gpsimd.collective_compute(
                "AllReduce", Alu.add, replica_groups=groups, ins=[contrib], outs=[contrib_o]
            )

            # ---------- anchors ----------
            canc = wk.tile([128, 128], F32, bufs=1)
            nc.sync.dma_start(canc[0:100, :], contrib_o[0:100, :])
            asq = wk.tile([128, 1], F32)
            ascr = wk.tile([128, 128], F32)
            nc.vector.scalar_tensor_tensor(out=ascr[0:100, :], in0=canc[0:100, :], scalar=1.0, in1=canc[0:100, :], op0=Alu.mult, op1=Alu.mult, accum_out=asq[0:100, :])
            nc.scalar.sqrt(asq[0:100, :], asq[0:100, :])
            nc.vector.tensor_scalar(out=asq[0:100, :], in0=asq[0:100, :], scalar1=1e-8, scalar2=None, op0=Alu.max)
            nc.vector.reciprocal(asq[0:100, :], asq[0:100, :])
            nc.vector.tensor_scalar(out=canc[0:100, :], in0=canc[0:100, :], scalar1=asq[0:100, :], scalar2=None, op0=Alu.mult)
            ancT_ps = simp.tile([128, 100], F32, tag="tsp")
            nc.tensor.transpose(ancT_ps[:], canc[0:100, :], ident[0:100, 0:100])
            ancT = wk.tile([128, 100], F32, bufs=1)
            nc.scalar.copy(ancT[:], ancT_ps[:])

            # ---------- memory tables: merge, normalize (1e-8), transpose ----------
            tabs = []
            for which, mem, cbase in ((0, pm, POS0), (1, nm, NEG0)):
                UT = big.tile([128, M], F32, name=f"UT{which}", tag=f"UT{which}")
                for i in range(8):
                    r0 = 128 * i
                    rn = min(128, M - r0)
                    mt = mem[i]
                    if r0 < KP:
                        newt = wk.tile([128, 128], F32, name=f"nw{which}{i}", tag="newt")
                        nc.sync.dma_start(newt[:], contrib_o[cbase + r0 : cbase + r0 + 128, :])
                        vcol = wk.tile([128, 1], F32, name=f"vc{which}{i}", tag="vcol")
                        nc.vector.tensor_scalar(out=vcol[:], in0=cntb[:, 1 + which : 2 + which], scalar1=float(-r0), scalar2=None, op0=Alu.add)
                        nc.vector.tensor_scalar(out=vcol[:], in0=rowiota[:], scalar1=vcol[:], scalar2=None, op0=Alu.is_lt)
                        vfull = wk.tile([128, 128], dt.uint8, name=f"vf{which}{i}", tag="vfull")
                        nc.vector.tensor_scalar(out=vfull[:], in0=ones_t[:], scalar1=vcol[:], scalar2=None, op0=Alu.mult)
                        nc.vector.copy_predicated(out=mt[:], mask=vfull[:], data=newt[:])
                    msq = wk.tile([128, 1], F32, name=f"msq{which}{i}", tag="msq")
                    mscr = wk.tile([128, 128], F32, name=f"mscr{which}{i}", tag="mscr")
                    nc.vector.scalar_tensor_tensor(out=mscr[0:rn, :], in0=mt[0:rn, :], scalar=1.0, in1=mt[0:rn, :], op0=Alu.mult, op1=Alu.mult, accum_out=msq[0:rn, :])
                    nc.scalar.sqrt(msq[0:rn, :], msq[0:rn, :])
                    nc.vector.tensor_scalar(out=msq[0:rn, :], in0=msq[0:rn, :], scalar1=1e-8, scalar2=None, op0=Alu.max)
                    nc.vector.reciprocal(msq[0:rn, :], msq[0:rn, :])
                    nc.vector.tensor_scalar(out=mt[0:rn, :], in0=mt[0:rn, :], scalar1=msq[0:rn, :], scalar2=None, op0=Alu.mult)
                    tp = simp.tile([128, 128], F32, name=f"tp{which}{i}", tag="tsp")
                    nc.tensor.transpose(tp[0:128, 0:rn], mt[0:rn, :], ident[0:rn, 0:rn])
                    nc.scalar.copy(UT[:, r0 : r0 + rn], tp[0:128, 0:rn])
                tabs.append(UT)
            U_posT, U_negT = tabs

            # ---------- sims ----------
            possim = simp.tile([100, M], F32)
            nc.tensor.matmul(possim[:, 0:512], ancT[:], U_posT[:, 0:512], start=True, stop=True)
            nc.tensor.matmul(possim[:, 512:1000], ancT[:], U_posT[:, 512:1000], start=True, stop=True)
            negsim = simp.tile([100, M], F32)
            nc.tensor.matmul(negsim[:, 0:512], ancT[:], U_negT[:, 0:512], start=True, stop=True)
            nc.tensor.matmul(negsim[:, 512:1000], ancT[:], U_negT[:, 512:1000], start=True, stop=True)
            nbuf = big.tile([100, M], F16)
            nc.scalar.mul(nbuf[:], negsim[:], -1.0)

            possim_sb = big.tile([100, M], F32)
            nc.scalar.copy(possim_sb[:], possim[:])
            nc.sync.dma_start(possim_d, possim_sb[:])
            poffreg = nc.values_load(poff_sb[0:1, 0:1].to_broadcast((1, 1)))
            mypos = wk.tile([100, PCOLS], F32, bufs=1)
            nc.sync.dma_start(mypos[:], possim_d[:, bass.ds(poffreg, PCOLS)])
            validA = wk.tile([128, 1], F32, bufs=1)
            nc.vector.tensor_scalar(out=validA[0:100, :], in0=rowiota[0:100, :], scalar1=cntb[0:100, 0:1], scalar2=None, op0=Alu.is_lt)
            amod = wk.tile([100, PCOLS], F32, bufs=1)
            nc.vector.tensor_scalar(out=amod[:], in0=mypos[:], scalar1=MARGIN + 4.0, scalar2=None, op0=Alu.add)
            nc.vector.tensor_scalar(out=amod[:], in0=amod[:], scalar1=validA[0:100, :], scalar2=4.0, op0=Alu.mult, op1=Alu.subtract)

            # ---------- pairwise relu-sum ----------
            zeros16 = big.tile([100, M], F16)
            nc.vector.memset(zeros16[:], 0.0)
            accD = wk.tile([100, 128], F32, bufs=1)
            nc.vector.memset(accD[:], 0.0)
            accA = wk.tile([100, 128], F32, bufs=1)
            nc.vector.memset(accA[:], 0.0)
            scrD = big.tile([100, M], F16)
            scrA = big.tile([100, M], F16)
            for i in range(PCOLS):
                if i < DVE_COLS:
                    nc.vector.scalar_tensor_tensor(
                        out=scrD[:], in0=nbuf[:], scalar=amod[:, i : i + 1], in1=zeros16[:],
                        op0=Alu.add, op1=Alu.max, accum_out=accD[:, i : i + 1],
                    )
                else:
                    nc.scalar.activation(
                        scrA[:], negsim[:], Act.Relu, bias=amod[:, i : i + 1], scale=-1.0,
                        accum_out=accA[:, i : i + 1],
                    )

            r1 = wk.tile([100, 2], F32, bufs=1)
            nc.vector.tensor_reduce(r1[:, 0:1], accD[:, 0:PCOLS], axis=mybir.AxisListType.X, op=Alu.add)
            nc.vector.tensor_reduce(r1[:, 1:2], accA[:, 0:PCOLS], axis=mybir.AxisListType.X, op=Alu.add)
            rsum = wk.tile([100, 1], F32, bufs=1)
            nc.vector.tensor_tensor(out=rsum[:], in0=r1[:, 0:1], in1=r1[:, 1:2], op=Alu.add)
            tot2 = ps.tile([1, 1], F32, tag="tiny")
            nc.tensor.matmul(tot2[:], rsum[:], ones_c[0:100, :], start=True, stop=True)
            tots = wk.tile([1, 1], F32, bufs=1)
            nc.scalar.copy(tots[:], tot2[:])
            den = wk.tile([1, 1], F32, bufs=1)
            nc.vector.tensor_scalar(out=den[:], in0=cntf[:, 0:1], scalar1=1.0, scalar2=1e6, op0=Alu.max, op1=Alu.mult)
            nc.vector.reciprocal(den[:], den[:])
            nc.vector.tensor_tensor(out=den[:], in0=den[:], in1=tots[:], op=Alu.mult)
            nc.sync.dma_start(out_d, den[:])

    return nc


def _host_shards(preds, embeddings, fsss_gts, pos_memory, neg_memory):
    """Build the 8 per-core input maps."""
    trils = np.tril(np.ones((128, 128), np.float32), -1).T  # lhsT[k,m]=1 iff k<m
    ident = np.eye(128, dtype=np.float32)
    rowiota = np.arange(128, dtype=np.float32).reshape(128, 1)
    riota1 = rowiota + 1.0
    siota3 = np.stack([np.arange(128, dtype=np.float32) + 128 * c for c in range(3)], axis=1)
    kvec = np.array([[KA, KP, KP, 0]], np.float32)

    in_maps = []
    for c in range(NCORES):
        psub = preds[c, :, ::4, ::4]  # [21,128,128]
        preds_t = np.ascontiguousarray(
            psub.transpose(1, 0, 2).reshape(128, C * 128)
        )
        gts_t = np.ascontiguousarray(fsss_gts[c, ::4, ::4]).astype(np.int32)
        embp = np.ascontiguousarray(
            embeddings[c].transpose(1, 2, 0).reshape(NPIX, D)
        )
        prefmask = np.zeros((8, 1), np.float32)
        prefmask[:c] = 1.0
        in_maps.append(
            {
                "preds_t": preds_t.astype(np.float32),
                "gts_t": gts_t,
                "embp": embp.astype(np.float32),
                "posmem": np.ascontiguousarray(pos_memory, dtype=np.float32),
                "negmem": np.ascontiguousarray(neg_memory, dtype=np.float32),
                "trils": trils.astype(np.float32),
                "ident": ident,
                "rowiota": rowiota,
                "riota1": riota1.astype(np.float32),
                "siota3": np.ascontiguousarray(siota3),
                "prefmask": prefmask,
                "kvec": kvec,
                "poff": np.array([[PCOLS * c]], np.int32),
            }
        )
    return in_maps


def kernel(preds, embeddings, fsss_gts, pos_memory, neg_memory):
    global LAST_EXEC_NS
    _install_patches()
    from concourse.bass_utils import run_bass_kernel_spmd

    if "nc" not in _cache:
        _cache["nc"] = _build_module()
    nc = _cache["nc"]

    in_maps = _host_shards(
        np.asarray(preds), np.asarray(embeddings), np.asarray(fsss_gts),
        np.asarray(pos_memory), np.asarray(neg_memory),
    )
    res = run_bass_kernel_spmd(nc, in_maps, list(range(NCORES)), trace=TRACE)
    LAST_EXEC_NS = res.exec_time_ns
    total = np.float32(0.0)
    for r in res.results:
        total = total + r["out"][0, 0]
    return np.float32(total)



# revision 12
# speedup vs baseline: 1.2132x; 1.2132x over previous
"""Trainium2 Bass kernel for nn_BoundaryLoss (8-core SPMD).

Self-contained: builds the Bass module once, shards full inputs across 8
NeuronCores (data-parallel over batch for the mask/gather stage; anchors x
pos x neg pairwise loss sharded by pos-memory columns), runs via
concourse.bass_utils.run_bass_kernel_spmd, and sums the per-core partial
losses on the host.

v2: dummy collective at t=0 absorbs the one-time comm-init barrier;
collective buffers are DRAM tile-pool tiles (dependency-tracked, fixes a
write/collective race in v1); contrib exchange in f16 (half traffic);
memory tables arrive host-transposed and are normalized via ones-matmul
column broadcasts; sims are single f16 matmuls; the pairwise relu loop
runs f16 on DVE (packed mode) + ScalarE with a measured split.
"""

import json
import sys
import types

import numpy as np

# ---------------------------------------------------------------------------
# Workaround 1: the walrus compiler in this container accepts only ONE sync
# wait per instruction; Tile's scheduler emits several on join points.  Split
# extra waits into standalone wait-only EventSemaphore instructions inserted
# right before the owning instruction (same engine, same block).
# ---------------------------------------------------------------------------


def _split_multiwaits_json(bir_bytes: bytes) -> bytes:
    j = json.loads(bir_bytes)
    ctr = 0
    changed = False
    for f in j.get("functions", []):
        for bb in f.get("blocks", []):
            new_insts = []
            for inst in bb.get("instructions", []):
                si = inst.get("sync_info")
                ow = (si or {}).get("on_wait") or []
                if len(ow) > 1:
                    changed = True
                    for w in ow[:-1]:
                        ctr += 1
                        new_insts.append(
                            {
                                "debug": inst.get("debug", 0),
                                "engine": inst["engine"],
                                "ins": [],
                                "outs": [],
                                "name": f"I-wsplit-{ctr}",
                                "opcode": "EventSemaphore",
                                "sync_info": {"on_update": [], "on_wait": [w]},
                            }
                        )
                    si["on_wait"] = [ow[-1]]
                new_insts.append(inst)
            bb["instructions"] = new_insts
    if not changed:
        return bir_bytes
    return json.dumps(j).encode()


_patched = False


def _install_patches():
    global _patched
    if _patched:
        return
    from concourse import bass as _bass

    _orig = _bass.Bass.to_json_bytes

    def _to_json_bytes(self):
        return _split_multiwaits_json(_orig(self))

    _bass.Bass.to_json_bytes = _to_json_bytes

    # Workaround 3: EVENT_SEMAPHORE_RANGE_CLEAR encodes a variable-length
    # struct this walrus only accepts for small ranges; clear in chunks.
    from concourse.bass import SemaphoreHandle as _SH, compact_to_ranges as _ctr

    def _clear_and_free(self, sems):
        if not sems:
            return
        sem_nums = [s.num if isinstance(s, _SH) else s for s in sems]
        for sem_range in _ctr(sem_nums):
            assert self._state.free_isdisjoint(sem_range)
            lo = sem_range.start
            while lo < sem_range.stop:
                hi = min(lo + 3, sem_range.stop)
                sub = range(lo, hi)
                self.gpsimd.dma_reset(sub)
                self.gpsimd.sem_clear(sub)
                lo = hi
        self._state.prepend_free_semaphores(sem_nums)
        for poison_set in self._tile_sem_poison_stack:
            poison_set.update(sem_nums)

    _bass.Bass.clear_and_free_semaphores = _clear_and_free

    # Workaround 2: the image lacks antenv.axon_hooks, so trace=True (NTFF
    # profiling) silently degrades.  Provide the module and register the
    # ctypes hook from trn_agent_boot if available.
    try:
        import antenv

        if "antenv.axon_hooks" not in sys.modules:
            m = types.ModuleType("antenv.axon_hooks")
            _store = {}
            m.set_axon_ntff_profile_hook = lambda h: _store.__setitem__("h", h)
            m.get_axon_ntff_profile_hook = lambda: _store.get("h")
            sys.modules["antenv.axon_hooks"] = m
            antenv.axon_hooks = m
            try:
                from trn_agent_boot.trn_boot import _ntff_profile_via_ctypes

                m.set_axon_ntff_profile_hook(
                    _ntff_profile_via_ctypes("/opt/axon/libaxon_pjrt.so")
                )
            except Exception:
                pass
    except Exception:
        pass
    _patched = True


# ---------------------------------------------------------------------------
# Problem constants (hardcoded from the spec)
# ---------------------------------------------------------------------------
B, C, H, W = 8, 21, 512, 512
hh = ww = 128
D = 128
M = 1000
KP = M // 3  # 333
KA = M // 10  # 100
MARGIN = 0.2
NPIX = hh * ww  # 16384 per core
NCORES = 8
PCOLS = M // NCORES  # 125 pos-columns per core

# contribution table layout (rows); windows allow a clamped global offset
# g0c<=K plus a full 384-slot (anchors: 128) local write.
ANC0 = 0
POS0 = 256
NEG0 = 976
CTOT = 1696

# (mask, chunk) pairs: anchors have 1 slot-chunk of 128, pos/neg 3 each.
CHUNKS = [(0, 0), (1, 0), (1, 1), (1, 2), (2, 0), (2, 1), (2, 2)]
NCH = len(CHUNKS)

DVE_COLS = 54  # pairwise columns on DVE; rest on ScalarE

TRACE = False
LAST_EXEC_NS = None

_cache = {}


def _build_module():
    from concourse import bass, tile
    import concourse.mybir as mybir

    dt = mybir.dt
    F32 = dt.float32
    F16 = dt.float16
    Alu = mybir.AluOpType
    Act = mybir.ActivationFunctionType
    AX = mybir.AxisListType.X

    nc = bass.Bass(
        trn_type="TRN2", target_bir_lowering=False, debug=False, num_devices=NCORES
    )

    # ---- I/O ----
    preds_t = nc.dram_tensor("preds_t", [128, C * 128], F32, kind="ExternalInput").ap()
    gts_t = nc.dram_tensor("gts_t", [128, 128], dt.int32, kind="ExternalInput").ap()
    embp = nc.dram_tensor("embp", [NPIX, D], F32, kind="ExternalInput").ap()
    pmT_in = nc.dram_tensor("pmT", [128, M], F32, kind="ExternalInput").ap()
    nmT_in = nc.dram_tensor("nmT", [128, M], F32, kind="ExternalInput").ap()
    trils_in = nc.dram_tensor("trils", [128, 128], F32, kind="ExternalInput").ap()
    ident_in = nc.dram_tensor("ident", [128, 128], F32, kind="ExternalInput").ap()
    rowiota_in = nc.dram_tensor("rowiota", [128, 1], F32, kind="ExternalInput").ap()
    riota1_in = nc.dram_tensor("riota1", [128, 1], F32, kind="ExternalInput").ap()
    siota3_in = nc.dram_tensor("siota3", [128, 3], F32, kind="ExternalInput").ap()
    prefmask_in = nc.dram_tensor("prefmask", [8, 1], F32, kind="ExternalInput").ap()
    kvec_in = nc.dram_tensor("kvec", [1, 4], F32, kind="ExternalInput").ap()
    iotar_in = nc.dram_tensor("iotar", [1, 512], F32, kind="ExternalInput").ap()
    poff_in = nc.dram_tensor("poff", [1, 1], dt.int32, kind="ExternalInput").ap()
    out_d = nc.dram_tensor("out", [1, 1], F32, kind="ExternalOutput").ap()

    groups = [list(range(NCORES))]

    with tile.TileContext(nc) as tc:
        with tc.tile_pool(name="dram", bufs=1, space="DRAM") as dpool, \
             tc.tile_pool(name="sb", bufs=1) as sb, \
             tc.tile_pool(name="pst", bufs=1, space="PSUM") as pst:

            # collective buffers as DRAM pool tiles => dependency-tracked
            dummy_i = dpool.tile([1, 1], F32, name="dummy_i")
            dummy_o = dpool.tile([1, 1], F32, name="dummy_o")
            cnt_loc = dpool.tile([1, 4], F32, name="cnt_loc")
            cnt_all = dpool.tile([8, 4], F32, name="cnt_all")
            contrib = dpool.tile([CTOT, D], F16, name="contrib")
            contrib_o = dpool.tile([CTOT, D], F16, name="contrib_o")
            possim_d = dpool.tile([100, M], F32, name="possim_d")

            # ---------- t=0: dummy collective absorbs the comm-init barrier
            zsm = sb.tile([1, 1], F32)
            nc.vector.memset(zsm[:], 0.0)
            nc.sync.dma_start(dummy_i[:], zsm[:])
            nc.gpsimd.collective_compute(
                "AllReduce", Alu.add, replica_groups=groups,
                ins=[dummy_i[:].opt()], outs=[dummy_o[:].opt()],
            )

            # ---------- constant / input loads ----------
            P_sb = sb.tile([128, C * 128], F32)
            nc.sync.dma_start(P_sb[:], preds_t)
            G = sb.tile([128, 128], dt.int32)
            nc.sync.dma_start(G[:], gts_t)
            PM = sb.tile([128, M], F32)
            nc.sync.dma_start(PM[:], pmT_in)
            NM = sb.tile([128, M], F32)
            nc.sync.dma_start(NM[:], nmT_in)
            trils = sb.tile([128, 128], F32)
            nc.sync.dma_start(trils[:], trils_in)
            ident = sb.tile([128, 128], F32)
            nc.sync.dma_start(ident[:], ident_in)
            rowiota = sb.tile([128, 1], F32)
            nc.sync.dma_start(rowiota[:], rowiota_in)
            riota1 = sb.tile([128, 1], F32)
            nc.sync.dma_start(riota1[:], riota1_in)
            siota3 = sb.tile([128, 3], F32)
            nc.sync.dma_start(siota3[:], siota3_in)
            prefmask = sb.tile([8, 1], F32)
            nc.sync.dma_start(prefmask[:], prefmask_in)
            kvec = sb.tile([1, 4], F32)
            nc.sync.dma_start(kvec[:], kvec_in)
            iotar = sb.tile([1, 512], F32)
            nc.sync.dma_start(iotar[:], iotar_in)
            poff_sb = sb.tile([1, 1], dt.int32)
            nc.sync.dma_start(poff_sb[:], poff_in)
            poffreg = nc.values_load(poff_sb[0:1, 0:1].to_broadcast((1, 1)))

            zeros = sb.tile([128, 128], F32)
            nc.vector.memset(zeros[:], 0.0)
            z16 = sb.tile([128, 128], F16)
            nc.vector.memset(z16[:], 0.0)
            ones_c = sb.tile([128, 1], F32)
            nc.vector.memset(ones_c[:], 1.0)
            ones_r = sb.tile([1, 128], F32)
            nc.vector.memset(ones_r[:], 1.0)

            # zero the f16 contribution table (14 chunk writes, off critical path)
            for i in range(13):
                nc.sync.dma_start(contrib[128 * i : 128 * (i + 1), :], z16[:])
            nc.sync.dma_start(contrib[1664:1696, :], z16[0:32, :])

            # ---------- masks (dense [128,128], f32 0/1) ----------
            mx = sb.tile([128, 128], F32)
            v = P_sb[:, 128 : C * 128].rearrange("p (c f) -> p f c", c=C - 1)
            nc.vector.tensor_reduce(mx[:], v, axis=AX, op=Alu.max)
            predm = sb.tile([128, 128], F32)
            nc.vector.tensor_tensor(out=predm[:], in0=mx[:], in1=P_sb[:, 0:128], op=Alu.is_gt)
            t1m = sb.tile([128, 128], F32)
            nc.vector.tensor_scalar(out=t1m[:], in0=G[:], scalar1=0.0, scalar2=None, op0=Alu.not_equal)
            t2m = sb.tile([128, 128], F32)
            nc.vector.tensor_scalar(out=t2m[:], in0=G[:], scalar1=255.0, scalar2=None, op0=Alu.not_equal)
            e0m = sb.tile([128, 128], F32)
            nc.vector.tensor_scalar(out=e0m[:], in0=G[:], scalar1=0.0, scalar2=None, op0=Alu.is_equal)
            gtm = sb.tile([128, 128], F32)
            nc.vector.tensor_tensor(out=gtm[:], in0=t1m[:], in1=t2m[:], op=Alu.mult)
            npredm = sb.tile([128, 128], F32)
            nc.vector.tensor_scalar(out=npredm[:], in0=predm[:], scalar1=-1.0, scalar2=1.0, op0=Alu.mult, op1=Alu.add)
            anc_m = sb.tile([128, 128], F32)
            nc.vector.tensor_tensor(out=anc_m[:], in0=predm[:], in1=gtm[:], op=Alu.mult)
            pos_m = sb.tile([128, 128], F32)
            nc.vector.tensor_tensor(out=pos_m[:], in0=gtm[:], in1=npredm[:], op=Alu.mult)
            neg_m = sb.tile([128, 128], F32)
            nc.vector.tensor_tensor(out=neg_m[:], in0=predm[:], in1=e0m[:], op=Alu.mult)
            masks = [anc_m, pos_m, neg_m]

            # ---------- local counts -> AllGather (kicked ASAP) ----------
            rs3 = sb.tile([128, 4], F32)
            nc.vector.memset(rs3[:], 0.0)
            for xi, mk in enumerate(masks):
                nc.vector.tensor_reduce(rs3[:, xi : xi + 1], mk[:], axis=AX, op=Alu.add)
            cnt_ps = pst.tile([1, 4], F32, tag="tiny")
            nc.tensor.matmul(cnt_ps[:], ones_c[:], rs3[:], start=True, stop=True)
            cnt_sb = sb.tile([1, 4], F32)
            nc.scalar.copy(cnt_sb[:], cnt_ps[:])
            nc.sync.dma_start(cnt_loc[:], cnt_sb[:])
            nc.gpsimd.collective_compute(
                "AllGather", Alu.bypass, replica_groups=groups,
                ins=[cnt_loc[:].opt()], outs=[cnt_all[:].opt()],
            )

            # ---------- selection: batched crossing search (local only) ----
            with tc.tile_pool(name="psel", bufs=1, space="PSUM") as psel:
                scns = []
                for xi, mk in enumerate(masks):
                    scn = sb.tile([128, 128], F32, name=f"scn{xi}")
                    nc.vector.tensor_tensor_scan(scn[:], mk[:], zeros[:], 0.0, Alu.add, Alu.add)
                    scns.append(scn)
                # per-mask exclusive prefix over partitions: rowoff3[:, m]
                last3 = sb.tile([128, 3], F32)
                for xi in range(3):
                    nc.vector.tensor_copy(last3[:, xi : xi + 1], scns[xi][:, 127:128])
                ro_ps = psel.tile([128, 3], F32, tag="sm")
                nc.tensor.matmul(ro_ps[:], trils[:], last3[:], start=True, stop=True)
                rowoff3 = sb.tile([128, 3], F32)
                nc.scalar.copy(rowoff3[:], ro_ps[:])
                # Pg = inclusive prefix + rowoff
                Pgs = []
                for xi in range(3):
                    Pg = sb.tile([128, 128], F32, name=f"Pg{xi}")
                    nc.vector.tensor_scalar(out=Pg[:], in0=scns[xi][:], scalar1=rowoff3[:, xi : xi + 1], scalar2=None, op0=Alu.add)
                    Pgs.append(Pg)
                # rowoffT [3,128]
                roT_ps = psel.tile([3, 128], F32, tag="sm2")
                nc.tensor.transpose(roT_ps[0:3, :], rowoff3[:, 0:3], ident[:])
                roT = sb.tile([3, 128], F32)
                nc.scalar.copy(roT[:], roT_ps[0:3, :])
                roTr = sb.tile([1, 3 * 128], F32)
                nc.sync.dma_start(roTr[:], roT[:])
                # RB7: per chunk j: broadcast rowoffT[mask] over partitions
                RB7_ps = psel.tile([128, NCH * 128], F32, tag="bigA")
                for j, (m, c) in enumerate(CHUNKS):
                    nc.tensor.matmul(RB7_ps[:, 128 * j : 128 * (j + 1)], ones_r[:], roTr[0:1, 128 * m : 128 * (m + 1)], start=True, stop=True)
                # cmp1: RB <= slotid (slotid = p + 128c); one op per chunk (PSUM in)
                cmp1 = sb.tile([128, NCH * 128], F32)
                for j, (m, c) in enumerate(CHUNKS):
                    nc.vector.tensor_scalar(out=cmp1[:, 128 * j : 128 * (j + 1)], in0=RB7_ps[:, 128 * j : 128 * (j + 1)], scalar1=siota3[:, c : c + 1], scalar2=None, op0=Alu.is_le)
                rc7 = sb.tile([128, NCH], F32)
                nc.vector.tensor_reduce(rc7[:], cmp1[:].rearrange("p (g f) -> p g f", g=NCH), axis=AX, op=Alu.add)
                # rcT [7,128]
                rcT_ps = psel.tile([NCH, 128], F32, tag="sm2")
                nc.tensor.transpose(rcT_ps[0:NCH, :], rc7[:, 0:NCH], ident[:])
                rcT = sb.tile([NCH, 128], F32)
                nc.scalar.copy(rcT[:], rcT_ps[0:NCH, :])
                rcTr = sb.tile([1, NCH * 128], F32)
                nc.sync.dma_start(rcTr[:], rcT[:])
                # rcb7 broadcast + Omat (row selector one-hot)
                rcb7_ps = psel.tile([128, NCH * 128], F32, tag="bigA")
                for j in range(NCH):
                    nc.tensor.matmul(rcb7_ps[:, 128 * j : 128 * (j + 1)], ones_r[:], rcTr[0:1, 128 * j : 128 * (j + 1)], start=True, stop=True)
                Omat7 = sb.tile([128, NCH * 128], F32)
                nc.vector.tensor_scalar(out=Omat7[:], in0=rcb7_ps[:], scalar1=riota1[:], scalar2=None, op0=Alu.is_equal)
                # prow = Omat^T @ Pg ; cmp2 = prow <= slotid ; wc = rowsum
                prow_ps = psel.tile([128, NCH * 128], F32, tag="bigB")
                for j, (m, c) in enumerate(CHUNKS):
                    nc.tensor.matmul(prow_ps[:, 128 * j : 128 * (j + 1)], Omat7[:, 128 * j : 128 * (j + 1)], Pgs[m][:], start=True, stop=True)
                cmp2 = sb.tile([128, NCH * 128], F32)
                for j, (m, c) in enumerate(CHUNKS):
                    nc.vector.tensor_scalar(out=cmp2[:, 128 * j : 128 * (j + 1)], in0=prow_ps[:, 128 * j : 128 * (j + 1)], scalar1=siota3[:, c : c + 1], scalar2=None, op0=Alu.is_le)
                wc7 = sb.tile([128, NCH], F32)
                nc.vector.tensor_reduce(wc7[:], cmp2[:].rearrange("p (g f) -> p g f", g=NCH), axis=AX, op=Alu.add)
                # idx = (rc-1)*128 + wc
                idx7 = sb.tile([128, NCH], F32)
                nc.vector.tensor_scalar(out=idx7[:], in0=rc7[:], scalar1=128.0, scalar2=-128.0, op0=Alu.mult, op1=Alu.add)
                nc.vector.tensor_tensor(out=idx7[:], in0=idx7[:], in1=wc7[:], op=Alu.add)
                idx7i = sb.tile([128, NCH], dt.int32)
                nc.vector.tensor_copy(idx7i[:], idx7[:])

                # ---------- gather + normalize (still counts-independent) --
                # canonical indirect-DMA form: one offset per PARTITION,
                # one gathered row per partition (see tile_scatter_add.py)
                gats = []
                for j, (m, c) in enumerate(CHUNKS):
                    gat = sb.tile([128, 128], F32, name=f"gat{j}")
                    nc.vector.memset(gat[:], 0.0)
                    nc.gpsimd.indirect_dma_start(
                        out=gat[:],
                        out_offset=None,
                        in_=embp,
                        in_offset=bass.IndirectOffsetOnAxis(ap=idx7i[:, j : j + 1], axis=0),
                        bounds_check=NPIX - 1,
                        oob_is_err=False,
                    )
                    gats.append(gat)
                ssq7 = sb.tile([128, NCH], F32)
                nscr = sb.tile([128, 128], F32)
                for j, (m, c) in enumerate(CHUNKS):
                    gv = gats[j][:]
                    nc.vector.scalar_tensor_tensor(out=nscr[:], in0=gv, scalar=1.0, in1=gv, op0=Alu.mult, op1=Alu.mult, accum_out=ssq7[:, j : j + 1])
                nc.scalar.sqrt(ssq7[:], ssq7[:])
                nc.vector.tensor_scalar(out=ssq7[:], in0=ssq7[:], scalar1=1e-12, scalar2=None, op0=Alu.max)
                nc.vector.reciprocal(ssq7[:], ssq7[:])
                gatn = []
                for j, (m, c) in enumerate(CHUNKS):
                    g16 = sb.tile([128, 128], F16, name=f"gatn{j}")
                    nc.vector.tensor_scalar(out=g16[:], in0=gats[j][:], scalar1=ssq7[:, j : j + 1], scalar2=None, op0=Alu.mult)
                    gatn.append(g16)

            # ---------- memtable pre-normalize of old cols [384:M] ---------
            with tc.tile_pool(name="psim", bufs=1, space="PSUM") as psim:
                UTp = sb.tile([128, M], F16, name="UTp")
                UTn = sb.tile([128, M], F16, name="UTn")
                OLDW = M - 384  # 616
                for which, PMt, UT in ((0, PM, UTp), (1, NM, UTn)):
                    sq = sb.tile([128, OLDW], F32, name=f"sqo{which}")
                    nc.vector.tensor_tensor(out=sq[:], in0=PMt[:, 384:M], in1=PMt[:, 384:M], op=Alu.mult)
                    csq_ps = psim.tile([1, OLDW], F32, name=f"csqo{which}", tag="scr")
                    nc.tensor.matmul(csq_ps[:, 0:512], ones_c[:], sq[:, 0:512], start=True, stop=True)
                    nc.tensor.matmul(csq_ps[:, 512:OLDW], ones_c[:], sq[:, 512:OLDW], start=True, stop=True)
                    invn = sb.tile([1, OLDW], F32, name=f"invno{which}")
                    nc.scalar.sqrt(invn[:], csq_ps[:])
                    nc.vector.tensor_scalar(out=invn[:], in0=invn[:], scalar1=1e-8, scalar2=None, op0=Alu.max)
                    nc.vector.reciprocal(invn[:], invn[:])
                    bc_ps = psim.tile([128, OLDW], F32, name=f"bco{which}", tag="scr")
                    nc.tensor.matmul(bc_ps[:, 0:512], ones_r[:], invn[:, 0:512], start=True, stop=True)
                    nc.tensor.matmul(bc_ps[:, 512:OLDW], ones_r[:], invn[:, 512:OLDW], start=True, stop=True)
                    nc.vector.tensor_tensor(out=UT[:, 384:M], in0=PMt[:, 384:M], in1=bc_ps[:], op=Alu.mult)

                # ---------- counts arrive: offsets ------------------------
                ca = sb.tile([8, 4], F32)
                nc.sync.dma_start(ca[:], cnt_all[:])
                g0_ps = pst.tile([1, 4], F32, tag="tiny")
                nc.tensor.matmul(g0_ps[:], prefmask[:], ca[:], start=True, stop=True)
                g0r = sb.tile([1, 4], F32)
                nc.scalar.copy(g0r[:], g0_ps[:])
                tot_ps = pst.tile([1, 4], F32, tag="tiny")
                nc.tensor.matmul(tot_ps[:], ones_c[0:8, :], ca[:], start=True, stop=True)
                totr = sb.tile([1, 4], F32)
                nc.scalar.copy(totr[:], tot_ps[:])
                cntf = sb.tile([1, 4], F32)  # final counts: min(total, k)
                nc.vector.tensor_tensor(out=cntf[:], in0=totr[:], in1=kvec[:], op=Alu.min)
                srow = sb.tile([1, 4], F32)  # S = clamp(k - g0, 0, 384)
                nc.vector.tensor_tensor(out=srow[:], in0=kvec[:], in1=g0r[:], op=Alu.subtract)
                nc.vector.tensor_scalar(out=srow[:], in0=srow[:], scalar1=0.0, scalar2=384.0, op0=Alu.max, op1=Alu.min)
                g0c = sb.tile([1, 4], F32)  # clamped g0
                nc.vector.tensor_tensor(out=g0c[:], in0=g0r[:], in1=kvec[:], op=Alu.min)
                g0c_i = sb.tile([1, 4], dt.int32)
                nc.vector.tensor_copy(g0c_i[:], g0c[:])
                cb_ps = pst.tile([128, 4], F32, tag="tiny")
                nc.tensor.matmul(cb_ps[:], ones_r[:], cntf[:], start=True, stop=True)
                cntb = sb.tile([128, 4], F32)
                nc.scalar.copy(cntb[:], cb_ps[:])
                sb_ps = pst.tile([128, 4], F32, tag="tiny")
                nc.tensor.matmul(sb_ps[:], ones_r[:], srow[:], start=True, stop=True)
                s128 = sb.tile([128, 4], F32)
                nc.scalar.copy(s128[:], sb_ps[:])

                # ---------- trim slots >= S, scatter to contrib (f16) ------
                for j, (m, c) in enumerate(CHUNKS):
                    vk = sb.tile([128, 1], F32, name=f"vk{j}")
                    nc.vector.tensor_scalar(out=vk[:], in0=siota3[:, c : c + 1], scalar1=s128[:, m : m + 1], scalar2=None, op0=Alu.is_lt)
                    nc.vector.tensor_scalar(out=gatn[j][:], in0=gatn[j][:], scalar1=vk[:], scalar2=None, op0=Alu.mult)
                BASES = {0: ANC0, 1: POS0, 2: NEG0}
                g0regs = {}
                for xi in range(3):
                    g0regs[xi] = nc.values_load(g0c_i[0:1, xi : xi + 1].to_broadcast((1, 1)))
                for j, (m, c) in enumerate(CHUNKS):
                    nc.sync.dma_start(contrib[bass.ds(g0regs[m] + (BASES[m] + 128 * c), 128), :], gatn[j][:])

                # ---------- AllReduce contributions (f16, 434KB) -----------
                nc.gpsimd.collective_compute(
                    "AllReduce", Alu.add, replica_groups=groups,
                    ins=[contrib[:].opt()], outs=[contrib_o[:].opt()],
                )

                # ---------- merge new rows into memtables, normalize [0:384]
                for which, base, PMt, UT in ((0, POS0, PM, UTp), (1, NEG0, NM, UTn)):
                    newT = sb.tile([128, 384], F16, name=f"newT{which}")
                    nc.sync.dma_start_transpose(newT[:], contrib_o[base : base + 384, :])
                    kcol = sb.tile([1, 384], F32, name=f"kcol{which}")
                    nc.vector.tensor_scalar(out=kcol[:], in0=iotar[0:1, 0:384], scalar1=cntf[0:1, 1 + which : 2 + which], scalar2=None, op0=Alu.is_lt)
                    km_ps = psim.tile([128, 384], F32, name=f"km{which}", tag="scr")
                    nc.tensor.matmul(km_ps[:], ones_r[:], kcol[:], start=True, stop=True)
                    vmask = sb.tile([128, 384], dt.uint8, name=f"vmask{which}")
                    nc.vector.tensor_copy(vmask[:], km_ps[:])
                    PMm = sb.tile([128, 384], F16, name=f"PMm{which}")
                    nc.vector.tensor_copy(PMm[:], PMt[:, 0:384])
                    nc.vector.copy_predicated(out=PMm[:], mask=vmask[:], data=newT[:])
                    sqm = sb.tile([128, 384], F32, name=f"sqm{which}")
                    nc.vector.tensor_tensor(out=sqm[:], in0=PMm[:], in1=PMm[:], op=Alu.mult)
                    csqm_ps = psim.tile([1, 384], F32, name=f"csqm{which}", tag="scr")
                    nc.tensor.matmul(csqm_ps[:], ones_c[:], sqm[:], start=True, stop=True)
                    invm = sb.tile([1, 384], F32, name=f"invm{which}")
                    nc.scalar.sqrt(invm[:], csqm_ps[:])
                    nc.vector.tensor_scalar(out=invm[:], in0=invm[:], scalar1=1e-8, scalar2=None, op0=Alu.max)
                    nc.vector.reciprocal(invm[:], invm[:])
                    bcm_ps = psim.tile([128, 384], F32, name=f"bcm{which}", tag="scr")
                    nc.tensor.matmul(bcm_ps[:], ones_r[:], invm[:], start=True, stop=True)
                    nc.vector.tensor_tensor(out=UT[:, 0:384], in0=PMm[:], in1=bcm_ps[:], op=Alu.mult)

                # ---------- anchors (already unit rows; f16) ---------------
                ancT = sb.tile([128, 128], F16)
                nc.sync.dma_start_transpose(ancT[:], contrib_o[0:128, :])

                # ---------- sims: single f16 matmuls -----------------------
                possim = psim.tile([100, M], F32, name="possim", tag="sims1")
                nc.tensor.matmul(possim[:, 0:512], ancT[:, 0:100], UTp[:, 0:512], start=True, stop=True)
                nc.tensor.matmul(possim[:, 512:M], ancT[:, 0:100], UTp[:, 512:M], start=True, stop=True)
                negsim = psim.tile([100, M], F32, name="negsim", tag="sims2")
                nc.tensor.matmul(negsim[:, 0:512], ancT[:, 0:100], UTn[:, 0:512], start=True, stop=True)
                nc.tensor.matmul(negsim[:, 512:M], ancT[:, 0:100], UTn[:, 512:M], start=True, stop=True)
                nbuf = sb.tile([100, M], F16)
                nc.scalar.mul(nbuf[:], negsim[:], -1.0)
                possim_sb = sb.tile([100, M], F32)
                nc.scalar.copy(possim_sb[:], possim[:])
                nc.sync.dma_start(possim_d[:], possim_sb[:])
                mypos = sb.tile([100, PCOLS], F32)
                nc.sync.dma_start(mypos[:], possim_d[:, bass.ds(poffreg, PCOLS)])
                validA = sb.tile([128, 1], F32)
                nc.vector.tensor_scalar(out=validA[0:100, :], in0=rowiota[0:100, :], scalar1=cntb[0:100, 0:1], scalar2=None, op0=Alu.is_lt)
                amod = sb.tile([100, PCOLS], F32)
                nc.vector.tensor_scalar(out=amod[:], in0=mypos[:], scalar1=MARGIN + 4.0, scalar2=None, op0=Alu.add)
                nc.vector.tensor_scalar(out=amod[:], in0=amod[:], scalar1=validA[0:100, :], scalar2=4.0, op0=Alu.mult, op1=Alu.subtract)

                # ---------- pairwise relu-sum ------------------------------
                accD = sb.tile([100, 128], F32)
                nc.vector.memset(accD[:], 0.0)
                accA = sb.tile([100, 128], F32)
                nc.vector.memset(accA[:], 0.0)
                scrD = sb.tile([100, M], F16)
                scrA = sb.tile([100, M], F16)
                zeros16 = sb.tile([100, M], F16)
                nc.vector.memset(zeros16[:], 0.0)
                for i in range(PCOLS):
                    if i < DVE_COLS:
                        nc.vector.scalar_tensor_tensor(
                            out=scrD[:], in0=nbuf[:], scalar=amod[:, i : i + 1], in1=zeros16[:],
                            op0=Alu.add, op1=Alu.max, accum_out=accD[:, i : i + 1],
                        )
                    else:
                        nc.scalar.activation(
                            scrA[:], negsim[:], Act.Relu, bias=amod[:, i : i + 1], scale=-1.0,
                            accum_out=accA[:, i : i + 1],
                        )

                r1 = sb.tile([100, 2], F32)
                nc.vector.tensor_reduce(r1[:, 0:1], accD[:], axis=AX, op=Alu.add)
                nc.vector.tensor_reduce(r1[:, 1:2], accA[:], axis=AX, op=Alu.add)
                rsum = sb.tile([100, 1], F32)
                nc.vector.tensor_tensor(out=rsum[:], in0=r1[:, 0:1], in1=r1[:, 1:2], op=Alu.add)
                tot2 = pst.tile([1, 1], F32, tag="tiny")
                nc.tensor.matmul(tot2[:], rsum[:], ones_c[0:100, :], start=True, stop=True)
                tots = sb.tile([1, 1], F32)
                nc.scalar.copy(tots[:], tot2[:])
                den = sb.tile([1, 1], F32)
                nc.vector.tensor_scalar(out=den[:], in0=cntf[:, 0:1], scalar1=1.0, scalar2=1e6, op0=Alu.max, op1=Alu.mult)
                nc.vector.reciprocal(den[:], den[:])
                nc.vector.tensor_tensor(out=den[:], in0=den[:], in1=tots[:], op=Alu.mult)
                nc.sync.dma_start(out_d, den[:])

    return nc


def _host_shards(preds, embeddings, fsss_gts, pos_memory, neg_memory):
    """Build the 8 per-core input maps (layout transforms only)."""
    trils = np.tril(np.ones((128, 128), np.float32), -1).T  # lhsT[k,m]=1 iff k<m
    ident = np.eye(128, dtype=np.float32)
    rowiota = np.arange(128, dtype=np.float32).reshape(128, 1)
    riota1 = rowiota + 1.0
    siota3 = np.stack([np.arange(128, dtype=np.float32) + 128 * c for c in range(3)], axis=1)
    kvec = np.array([[KA, KP, KP, 0]], np.float32)
    iotar = np.arange(512, dtype=np.float32).reshape(1, 512)
    pmT = np.ascontiguousarray(pos_memory.T, dtype=np.float32)
    nmT = np.ascontiguousarray(neg_memory.T, dtype=np.float32)

    in_maps = []
    for c in range(NCORES):
        psub = preds[c, :, ::4, ::4]  # [21,128,128]
        preds_t = np.ascontiguousarray(
            psub.transpose(1, 0, 2).reshape(128, C * 128)
        )
        gts_t = np.ascontiguousarray(fsss_gts[c, ::4, ::4]).astype(np.int32)
        embp = np.ascontiguousarray(
            embeddings[c].transpose(1, 2, 0).reshape(NPIX, D)
        )
        prefmask = np.zeros((8, 1), np.float32)
        prefmask[:c] = 1.0
        in_maps.append(
            {
                "preds_t": preds_t.astype(np.float32),
                "gts_t": gts_t,
                "embp": embp.astype(np.float32),
                "pmT": pmT,
                "nmT": nmT,
                "trils": trils.astype(np.float32),
                "ident": ident,
                "rowiota": rowiota,
                "riota1": riota1.astype(np.float32),
                "siota3": np.ascontiguousarray(siota3),
                "prefmask": prefmask,
                "kvec": kvec,
                "iotar": iotar,
                "poff": np.array([[PCOLS * c]], np.int32),
            }
        )
    return in_maps


def kernel(preds, embeddings, fsss_gts, pos_memory, neg_memory):
    global LAST_EXEC_NS
    _install_patches()
    from concourse.bass_utils import run_bass_kernel_spmd

    if "nc" not in _cache:
        _cache["nc"] = _build_module()
    nc = _cache["nc"]

    in_maps = _host_shards(
        np.asarray(preds), np.asarray(embeddings), np.asarray(fsss_gts),
        np.asarray(pos_memory), np.asarray(neg_memory),
    )
    res = run_bass_kernel_spmd(nc, in_maps, list(range(NCORES)), trace=TRACE)
    LAST_EXEC_NS = res.exec_time_ns
    total = np.float32(0.0)
    for r in res.results:
        total = total + r["out"][0, 0]
    return np.float32(total)
